# revision 45
# baseline (speedup 1.0000x reference)
"""DETR-style DetectionLoss on 8 Trainium2 NeuronCores.

Pipeline:
  1. Host: softmax(pred_logits), shard samples across 8 cores balanced by
     valid-gt count, pack valid gts of several samples into 128-partition
     "groups" (<=7 sample-slices per group).
  2. Device (per core, Bass/Tile): for each group, compute the matching cost
     matrix rows  F[g, q] = cls + 2.5*l1 - iou - U/enc  (an affine transform
     of the reference cost, which leaves the Hungarian assignment invariant).
     Pred-side per-query rows are broadcast across gt partitions with one
     K=28 bf16 matmul per block: exact 0/1 indicator weights for the hi/lo
     bf16-split pred rows, plus hi/lo-split gt bias value rows, so each psum
     block is fully formed at ~1e-6 accuracy (this stack's fp32 matmul is
     only ~6e-4 accurate). Elementwise GIoU/L1 runs in bf16 (validated: bf16
     noise on the cost only shifts the final loss ~2e-4), batched 512-wide
     over psum-chunk pairs, spread across Act (|.| + exp(-ln) reciprocals),
     DVE, and GpSimd.
  3. Host: Jonker-Volgenant assignment per sample (float64, same algorithm
     as the reference), then the final scalar loss (CE + L1 + GIoU).
"""

import os
import sys

import numpy as np

for _p in ("/opt/trn_rl_repo", "/root/.axon_site/_ro/trn_rl_repo"):
    if os.path.isdir(_p) and _p not in sys.path:
        sys.path.append(_p)

import ml_dtypes  # noqa: E402
import concourse.bass as bass  # noqa: E402
import concourse.mybir as mybir  # noqa: E402
from concourse.tile import TileContext  # noqa: E402
from concourse.bass_utils import run_bass_kernel_spmd  # noqa: E402
from concourse.alu_op_type import AluOpType  # noqa: E402

BF16 = ml_dtypes.bfloat16
AF = mybir.ActivationFunctionType

NUM_CLASSES = 10
CLS_W, L1_W, GIOU_W = 2.0, 5.0, 2.0
B, Q, G, C1 = 64, 900, 100, NUM_CLASSES + 1
NCORES = 8
QPAD = 1024
QC = 256
# chunk column ranges (last chunk only covers the 900 real queries)
CHUNKS = [(0, 256), (256, 512), (512, 768), (768, 900)]
NBLK = 7  # psum block order: C1, W, C2, H, Sx, Sy, A
KROWS = 28  # 7 ind-hi + 7 ind-lo + 7 bias-hi + 7 bias-lo
MAX_SLICES = 7

# ----------------------------------------------------------------- walrus fix


def _split_multiwait(bir_json: bytes) -> bytes:
    """This walrus build accepts at most 1 sync-wait per instruction on
    several encodings; split extra waits onto preceding same-engine Drains."""
    import json

    j = json.loads(bir_json)
    changed = False
    for fn in j.get("functions", []):
        for bb in fn.get("blocks", []):
            out = []
            for ins in bb.get("instructions", []):
                si = ins.get("sync_info") or {}
                w = si.get("on_wait") or []
                if len(w) > 1:
                    for ci, wi in enumerate(w[:-1]):
                        out.append({
                            "name": ins["name"] + f"-wsplit{ci}",
                            "opcode": "Drain",
                            "engine": ins.get("engine", "SP"),
                            "ins": [],
                            "outs": [],
                            "is_reset_sema": False,
                            "debug": ins.get("debug", 0),
                            "sync_info": {"on_update": [], "on_wait": [wi]},
                        })
                        changed = True
                    ins["sync_info"]["on_wait"] = [w[-1]]
                out.append(ins)
            bb["instructions"] = out
    return json.dumps(j).encode() if changed else bir_json


_patched = False


def _install_patch():
    global _patched
    if _patched:
        return
    _patched = True
    import concourse.bass_utils as bu
    import concourse.bass2jax as b2j

    orig = bu.compile_bir_kernel

    def patched(bir_json, tmpdir, neff_name="file.neff"):
        return orig(_split_multiwait(bir_json), tmpdir, neff_name)

    bu.compile_bir_kernel = patched
    b2j.compile_bir_kernel = patched


# ------------------------------------------------------------------ device


_KERNEL_CACHE = {}


def _build_nc(ngroups: int):
    """Blocks (psum, fully formed incl. gt-side bias): order [C1 W C2 H Sx Sy A]
      C1 = 2.5*Dcx, W = 1.25*Dw, C2 = 2.5*Dcy, H = 1.25*Dh,
      Sx = 1.25*(pw+gw), Sy = 1.25*(ph+gh), A = 6.25*(pa+ga)
    a_all = |[C1 W C2 H]|;  P = max(a1, aw), Q = max(a2, ah)   (2.5*P_t)
    iw,ih = relu([Sx Sy] - [P Q]); ew,eh = [Sx Sy] + [P Q]
    inter = iw*ih (6.25x), enc = ew*eh, U = A - inter
    [rU rE] = exp(-ln([U enc]));  [iou vv] = [inter U] * [rU rE]
    l1t = (a1+a2) + 2*(aw+ah)   (2.5*l1_t)
    F = CLS + l1t - iou - vv"""
    nc = bass.Bass(target_bir_lowering=False)
    f32, bf16 = mybir.dt.float32, mybir.dt.bfloat16
    lhsT = nc.dram_tensor("lhsT", [ngroups, KROWS, 128], bf16,
                          kind="ExternalInput")
    rhs = nc.dram_tensor("rhs", [ngroups, KROWS, NBLK, QPAD], bf16,
                         kind="ExternalInput")
    cls = nc.dram_tensor("cls", [ngroups, 128, QPAD], bf16,
                         kind="ExternalInput")
    out = nc.dram_tensor("out", [ngroups, 128, QPAD], bf16,
                         kind="ExternalOutput")

    def pv(ap2d, nb, w):
        """[128, nb*QC] slab -> 3D AP [128, nb, w] over the first w cols of
        each QC-wide slot."""
        v = ap2d.rearrange("p (b q) -> p b q", b=nb)
        return v[:, :, :w] if w != QC else v

    with TileContext(nc) as tc:
        with (
            tc.tile_pool(name="io", bufs=3) as io,
            tc.tile_pool(name="mid", bufs=4) as mid,
            tc.tile_pool(name="ps", bufs=2, space="PSUM") as psp,
        ):
            SL = 2 * QC  # 512-wide slots: elementwise ops batch chunk pairs

            def sv(ap2d, nb, w):
                v = ap2d.rearrange("p (b q) -> p b q", b=nb)
                return v[:, :, :w] if w != SL else v

            for g in range(ngroups):
                lt = io.tile([KROWS, 128], bf16, tag="lt", name="lt")
                rt = io.tile([KROWS, NBLK * QPAD], bf16, tag="rt", name="rt")
                ct = io.tile([128, QPAD], bf16, tag="ct", name="ct")
                nc.sync.dma_start(out=lt, in_=lhsT[g])
                rtv = rt.rearrange("k (b q) -> k b q", b=NBLK)
                nc.sync.dma_start(out=rtv[:, :, 0:512],
                                  in_=rhs[g][:, :, 0:512])
                nc.sync.dma_start(out=rtv[:, :, 512:],
                                  in_=rhs[g][:, :, 512:])
                nc.sync.dma_start(out=ct[:, 0:512], in_=cls[g][:, 0:512])
                nc.sync.dma_start(out=ct[:, 512:], in_=cls[g][:, 512:])
                rt3 = rt.rearrange("k (b q) -> k b q", b=NBLK)
                for (p0, p1) in ((0, 512), (512, 900)):
                    w = p1 - p0  # live width of this pair (512 or 388)

                    def t(tag, wdt=1):
                        return mid.tile([128, wdt * SL], bf16, tag=tag,
                                        name=tag)

                    # two 256-col psum chunks feed one 512-wide sbuf pair
                    aall = t("aall", 7)
                    a7 = aall.rearrange("p (b q) -> p b q", b=7)
                    for sub in (0, QC):
                        c0 = p0 + sub
                        c1 = min(c0 + QC, 900)
                        ws = c1 - c0
                        ps = psp.tile([128, NBLK * QC], f32, tag="ps",
                                      name="ps")
                        for j in range(NBLK):
                            nc.tensor.matmul(ps[:, j * QC:j * QC + ws],
                                             lt[:, :], rt3[:, j, c0:c1],
                                             start=True, stop=True)
                        # |.| of all 7 blocks in one op; [Sx Sy A] are
                        # positive so Abs doubles as the psum->bf16 copy.
                        nc.scalar.activation(
                            out=a7[:, :, sub:sub + ws],
                            in_=pv(ps[:, :7 * QC], 7, ws),
                            func=AF.Abs)
                    a1 = aall[:, 0 * SL:0 * SL + w]
                    aw_ = aall[:, 1 * SL:1 * SL + w]
                    a2 = aall[:, 2 * SL:2 * SL + w]
                    ah = aall[:, 3 * SL:3 * SL + w]
                    pq = t("pq", 2)
                    nc.vector.tensor_tensor(out=pq[:, :w], in0=a1, in1=aw_,
                                            op=AluOpType.max)
                    nc.vector.tensor_tensor(out=pq[:, SL:SL + w], in0=a2,
                                            in1=ah, op=AluOpType.max)
                    sxy = sv(aall[:, 4 * SL:6 * SL], 2, w)
                    pqv = sv(pq, 2, w)
                    dd = t("dd", 2)
                    nc.vector.tensor_tensor(out=sv(dd, 2, w), in0=sxy,
                                            in1=pqv, op=AluOpType.subtract)
                    ii = t("ii", 2)
                    nc.vector.tensor_scalar_max(out=sv(ii, 2, w),
                                                in0=sv(dd, 2, w), scalar1=0.0)
                    ee = t("ee", 2)
                    nc.vector.tensor_tensor(out=sv(ee, 2, w), in0=sxy,
                                            in1=pqv, op=AluOpType.add)
                    iue = t("iue", 3)  # slots: inter, U, enc
                    inter = iue[:, 0 * SL:0 * SL + w]
                    uslot = iue[:, 1 * SL:1 * SL + w]
                    encs = iue[:, 2 * SL:2 * SL + w]
                    nc.vector.tensor_tensor(out=inter, in0=ii[:, :w],
                                            in1=ii[:, SL:SL + w],
                                            op=AluOpType.mult)
                    nc.vector.tensor_tensor(out=encs, in0=ee[:, :w],
                                            in1=ee[:, SL:SL + w],
                                            op=AluOpType.mult)
                    nc.vector.tensor_tensor(
                        out=uslot, in0=aall[:, 6 * SL:6 * SL + w],
                        in1=inter, op=AluOpType.subtract)
                    rr = t("rr", 2)
                    nc.scalar.activation(out=sv(rr, 2, w),
                                         in_=sv(iue[:, SL:3 * SL], 2, w),
                                         func=AF.Ln)
                    nc.scalar.activation(out=sv(rr, 2, w), in_=sv(rr, 2, w),
                                         func=AF.Exp, scale=-1.0)
                    iv = t("iv", 2)
                    nc.vector.tensor_tensor(out=sv(iv, 2, w),
                                            in0=sv(iue[:, 0:2 * SL], 2, w),
                                            in1=sv(rr, 2, w),
                                            op=AluOpType.mult)
                    # l1t = (a1+a2) + 2*(aw+ah): s1 = [a1+a2 | aw+ah]
                    s1 = t("s1", 2)
                    nc.vector.tensor_tensor(out=sv(s1, 2, w),
                                            in0=sv(aall[:, 0:2 * SL], 2, w),
                                            in1=sv(aall[:, 2 * SL:4 * SL], 2, w),
                                            op=AluOpType.add)
                    # l1t = s1a + 2*s1b as two Pool adds (off critical path)
                    l1h = t("l1h")
                    nc.gpsimd.tensor_tensor(out=l1h[:, :w], in0=s1[:, :w],
                                            in1=s1[:, SL:SL + w],
                                            op=AluOpType.add)
                    l1t = t("l1t")
                    nc.gpsimd.tensor_tensor(out=l1t[:, :w], in0=l1h[:, :w],
                                            in1=s1[:, SL:SL + w],
                                            op=AluOpType.add)
                    zc, t4, ff = t("zc"), t("t4"), t("ff")
                    nc.gpsimd.tensor_tensor(out=zc[:, :w], in0=l1t[:, :w],
                                            in1=ct[:, p0:p1],
                                            op=AluOpType.add)
                    nc.vector.tensor_tensor(out=t4[:, :w], in0=iv[:, :w],
                                            in1=iv[:, SL:SL + w],
                                            op=AluOpType.add)
                    nc.gpsimd.tensor_tensor(out=ff[:, :w], in0=zc[:, :w],
                                            in1=t4[:, :w],
                                            op=AluOpType.subtract)
                    nc.sync.dma_start(out=out[g][:, p0:p1], in_=ff[:, :w])
    return nc


def _get_nc(ngroups: int):
    if ngroups not in _KERNEL_CACHE:
        _KERNEL_CACHE[ngroups] = _build_nc(ngroups)
    return _KERNEL_CACHE[ngroups]


# ------------------------------------------------------------------- host


def _hungarian(cost):
    # Jonker-Volgenant shortest-augmenting-path (mirrors the reference).
    cost = np.asarray(cost, dtype=np.float64)
    transposed = cost.shape[0] > cost.shape[1]
    C = cost.T if transposed else cost
    n, m = C.shape
    u = np.zeros(n + 1)
    v = np.zeros(m + 1)
    p = np.zeros(m + 1, dtype=np.int64)
    colr = np.arange(m + 1)
    for i in range(1, n + 1):
        p[0] = i
        j0 = 0
        minv = np.full(m + 1, np.inf)
        way = np.zeros(m + 1, dtype=np.int64)
        used = np.zeros(m + 1, dtype=bool)
        while True:
            used[j0] = True
            i0 = p[j0]
            unused = colr[1:][~used[1:]]
            cur = C[i0 - 1, unused - 1] - u[i0] - v[unused]
            better = cur < minv[unused]
            bidx = unused[better]
            minv[bidx] = cur[better]
            way[bidx] = j0
            j1 = unused[np.argmin(minv[unused])]
            delta = minv[j1]
            u[p[used]] += delta
            v[used] -= delta
            minv[~used] -= delta
            j0 = j1
            if p[j0] == 0:
                break
        while j0 != 0:
            jn = way[j0]
            p[j0] = p[jn]
            j0 = jn
    rows = p[1:] - 1
    cols = np.arange(m)
    sel = rows >= 0
    r, c = rows[sel], cols[sel]
    if transposed:
        r, c = c, r
    order = np.argsort(r)
    return r[order], c[order]


def _softmax_f32(x):
    x = x.astype(np.float32)
    m = x.max(axis=-1, keepdims=True)
    e = np.exp(x - m)
    return e / e.sum(axis=-1, keepdims=True)


def _bf16_split(x):
    hi = x.astype(np.float32).astype(BF16)
    lo = (x.astype(np.float32) - hi.astype(np.float32)).astype(BF16)
    return hi, lo


def _plan(nvalid):
    """Assign samples to cores (balanced by nvalid) and first-fit-decreasing
    pack whole samples into groups of <=128 gt rows / <=MAX_SLICES slices.
    Samples are never split across groups (the per-sample Hungarian needs
    the full cost matrix in one group)."""
    order = np.argsort(-nvalid, kind="stable")
    loads = [0] * NCORES
    core_samples = [[] for _ in range(NCORES)]
    for b in order:
        if nvalid[b] == 0:
            continue
        c = int(np.argmin(loads))
        loads[c] += int(nvalid[b])
        core_samples[c].append(int(b))
    plans = []
    for c in range(NCORES):
        bins = []  # list of [rows_used, slices]
        for b in core_samples[c]:  # already descending by nvalid
            n = int(nvalid[b])
            for bin_ in bins:
                if bin_[0] + n <= 128 and len(bin_[1]) < MAX_SLICES:
                    bin_[1].append((b, 0, n, bin_[0]))
                    bin_[0] += n
                    break
            else:
                bins.append([n, [(b, 0, n, 0)]])
        plans.append([bin_[1] for bin_ in bins])
    return plans


def kernel(pred_logits, pred_boxes, gt_classes, gt_boxes):
    _install_patch()
    pred_logits = np.asarray(pred_logits)
    pred_boxes = np.asarray(pred_boxes, dtype=np.float32)
    gt_classes = np.asarray(gt_classes)
    gt_boxes = np.asarray(gt_boxes, dtype=np.float32)
    Bv, Qv = pred_logits.shape[:2]

    prob = _softmax_f32(pred_logits)  # (B, Q, C+1)
    valid = gt_classes >= 0
    nvalid = valid.sum(1).astype(np.int64)
    plans = _plan(nvalid)
    ngroups = max((len(p) for p in plans), default=0)

    b_f = p_f = g_f = np.zeros(0, np.int64)
    if ngroups > 0:
        # pred-side rows per block, shared per sample; block order
        # [C1 W C2 H Sx Sy A]
        pcx, pcy = pred_boxes[..., 0], pred_boxes[..., 1]
        pw, ph = pred_boxes[..., 2], pred_boxes[..., 3]
        pa = pw * ph
        pred_rows = np.stack(  # (B, NBLK, Q)
            [2.5 * pcx, 1.25 * pw, 2.5 * pcy, 1.25 * ph,
             1.25 * pw, 1.25 * ph, 6.25 * pa], axis=1)
        # gt-side bias value per block (full G; sliced per group)
        gcx, gcy = gt_boxes[..., 0], gt_boxes[..., 1]
        gw, gh = gt_boxes[..., 2], gt_boxes[..., 3]
        ga = gw * gh
        bias_all = np.stack(  # (B, NBLK, G)
            [-2.5 * gcx, -1.25 * gw, -2.5 * gcy, -1.25 * gh,
             1.25 * gw, 1.25 * gh, 6.25 * ga], axis=1).astype(np.float32)

        lhsT = np.zeros((NCORES, ngroups, KROWS, 128), BF16)
        rhs = np.zeros((NCORES, ngroups, KROWS, NBLK, QPAD), BF16)
        clsb = np.zeros((NCORES, ngroups, 128, QPAD), BF16)
        pr_hi = {}
        for c in range(NCORES):
            for g, slices in enumerate(plans[c]):
                for si, (b, vstart, ln, poff) in enumerate(slices):
                    vidx = np.nonzero(valid[b])[0][vstart:vstart + ln]
                    if b not in pr_hi:
                        pr_hi[b] = _bf16_split(pred_rows[b])
                    hi, lo = pr_hi[b]  # (NBLK, Q)
                    lhsT[c, g, si, poff:poff + ln] = 1.0
                    lhsT[c, g, 7 + si, poff:poff + ln] = 1.0
                    rhs[c, g, si, :, :Qv] = hi
                    rhs[c, g, 7 + si, :, :Qv] = lo
                    bhi, blo = _bf16_split(bias_all[b, :, vidx])  # (ln, NBLK)
                    lhsT[c, g, 14:21, poff:poff + ln] = bhi.T
                    lhsT[c, g, 21:28, poff:poff + ln] = blo.T
                    clsb[c, g, poff:poff + ln, :Qv] = \
                        -prob[b][:, gt_classes[b, vidx]].T
                for j in range(NBLK):
                    rhs[c, g, 14 + j, j, :] = 1.0
                    rhs[c, g, 21 + j, j, :] = 1.0
        in_maps = [{"lhsT": lhsT[c], "rhs": rhs[c],
                    "cls": clsb[c]} for c in range(NCORES)]
        nc = _get_nc(ngroups)
        res = run_bass_kernel_spmd(nc, in_maps, core_ids=list(range(NCORES)))

        bs, ps_, gs = [], [], []
        for c in range(NCORES):
            outc = res.results[c]["out"]  # (ngroups, 128, QPAD)
            for g, slices in enumerate(plans[c]):
                for (b, vstart, ln, poff) in slices:
                    Fm = outc[g, poff:poff + ln, :Qv].astype(np.float32)
                    vidx = np.nonzero(valid[b])[0][vstart:vstart + ln]
                    r, q = _hungarian(Fm)  # rows=gt-slice, cols=query
                    bs.append(np.full(r.size, b, dtype=np.int64))
                    ps_.append(q.astype(np.int64))
                    gs.append(vidx[r].astype(np.int64))
        if bs:
            b_f = np.concatenate(bs)
            p_f = np.concatenate(ps_)
            g_f = np.concatenate(gs)

    # --- final loss on host (mirrors reference.detection_loss) ---
    n_matched = max(int(b_f.size), 1)
    tc = np.full((Bv, Qv), NUM_CLASSES, dtype=np.int64)
    if b_f.size:
        tc[b_f, p_f] = gt_classes[b_f, g_f]
    logits = pred_logits.astype(np.float64)
    m = logits.max(-1, keepdims=True)
    lse = m + np.log(np.exp(logits - m).sum(-1, keepdims=True))
    logp = (logits - lse).reshape(-1, C1)
    tt = tc.reshape(-1)
    w = np.ones(C1)
    w[-1] = 0.1
    nll = -logp[np.arange(tt.size), tt]
    wt = w[tt]
    cls_loss = (wt * nll).sum() / wt.sum()
    if b_f.size:
        mp = pred_boxes[b_f, p_f].astype(np.float64)
        mg = gt_boxes[b_f, g_f].astype(np.float64)
        l1 = np.abs(mp - mg).sum()
        mpx = np.stack([mp[:, 0] - mp[:, 2] / 2, mp[:, 1] - mp[:, 3] / 2,
                        mp[:, 0] + mp[:, 2] / 2, mp[:, 1] + mp[:, 3] / 2], -1)
        mgx = np.stack([mg[:, 0] - mg[:, 2] / 2, mg[:, 1] - mg[:, 3] / 2,
                        mg[:, 0] + mg[:, 2] / 2, mg[:, 1] + mg[:, 3] / 2], -1)
        a1 = np.clip(mpx[:, 2] - mpx[:, 0], 0, None) * \
            np.clip(mpx[:, 3] - mpx[:, 1], 0, None)
        a2 = np.clip(mgx[:, 2] - mgx[:, 0], 0, None) * \
            np.clip(mgx[:, 3] - mgx[:, 1], 0, None)
        lt = np.maximum(mpx[:, :2], mgx[:, :2])
        rb = np.minimum(mpx[:, 2:], mgx[:, 2:])
        wh = np.clip(rb - lt, 0, None)
        inter = wh[:, 0] * wh[:, 1]
        union = a1 + a2 - inter
        iou = inter / np.clip(union, 1e-6, None)
        lte = np.minimum(mpx[:, :2], mgx[:, :2])
        rbe = np.maximum(mpx[:, 2:], mgx[:, 2:])
        whe = np.clip(rbe - lte, 0, None)
        encl = whe[:, 0] * whe[:, 1]
        gd = iou - (encl - union) / np.clip(encl, 1e-6, None)
        gl = (1.0 - gd).sum()
    else:
        l1 = 0.0
        gl = 0.0
    total = CLS_W * cls_loss + L1_W * l1 / n_matched + GIOU_W * gl / n_matched
    return np.float32(total)


# revision 46
# speedup vs baseline: 1.0054x; 1.0054x over previous
"""DETR-style DetectionLoss on 8 Trainium2 NeuronCores.

Pipeline:
  1. Host: softmax(pred_logits), shard samples across 8 cores balanced by
     valid-gt count, pack valid gts of several samples into 128-partition
     "groups" (<=7 sample-slices per group).
  2. Device (per core, Bass/Tile): for each group, compute the matching cost
     matrix rows  F[g, q] = cls + 2.5*l1 - iou - U/enc  (an affine transform
     of the reference cost, which leaves the Hungarian assignment invariant).
     Pred-side per-query rows are broadcast across gt partitions with one
     K=28 bf16 matmul per block: exact 0/1 indicator weights for the hi/lo
     bf16-split pred rows, plus hi/lo-split gt bias value rows, so each psum
     block is fully formed at ~1e-6 accuracy (this stack's fp32 matmul is
     only ~6e-4 accurate). Elementwise GIoU/L1 runs in bf16 (validated: bf16
     noise on the cost only shifts the final loss ~2e-4), batched 512-wide
     over psum-chunk pairs, spread across Act (|.| + exp(-ln) reciprocals),
     DVE, and GpSimd.
  3. Host: Jonker-Volgenant assignment per sample (float64, same algorithm
     as the reference), then the final scalar loss (CE + L1 + GIoU).
"""

import os
import sys

import numpy as np

for _p in ("/opt/trn_rl_repo", "/root/.axon_site/_ro/trn_rl_repo"):
    if os.path.isdir(_p) and _p not in sys.path:
        sys.path.append(_p)

import ml_dtypes  # noqa: E402
import concourse.bass as bass  # noqa: E402
import concourse.mybir as mybir  # noqa: E402
from concourse.tile import TileContext  # noqa: E402
from concourse.bass_utils import run_bass_kernel_spmd  # noqa: E402
from concourse.alu_op_type import AluOpType  # noqa: E402

BF16 = ml_dtypes.bfloat16
AF = mybir.ActivationFunctionType

NUM_CLASSES = 10
CLS_W, L1_W, GIOU_W = 2.0, 5.0, 2.0
B, Q, G, C1 = 64, 900, 100, NUM_CLASSES + 1
NCORES = 8
QPAD = 1024
QC = 256
# chunk column ranges (last chunk only covers the 900 real queries)
CHUNKS = [(0, 256), (256, 512), (512, 768), (768, 900)]
NBLK = 7  # psum block order: C1, W, C2, H, Sx, Sy, A
KROWS = 28  # 7 ind-hi + 7 ind-lo + 7 bias-hi + 7 bias-lo
MAX_SLICES = 7

# ----------------------------------------------------------------- walrus fix


def _split_multiwait(bir_json: bytes) -> bytes:
    """This walrus build accepts at most 1 sync-wait per instruction on
    several encodings; split extra waits onto preceding same-engine Drains."""
    import json

    j = json.loads(bir_json)
    changed = False
    for fn in j.get("functions", []):
        for bb in fn.get("blocks", []):
            out = []
            for ins in bb.get("instructions", []):
                si = ins.get("sync_info") or {}
                w = si.get("on_wait") or []
                if len(w) > 1:
                    for ci, wi in enumerate(w[:-1]):
                        out.append({
                            "name": ins["name"] + f"-wsplit{ci}",
                            "opcode": "Drain",
                            "engine": ins.get("engine", "SP"),
                            "ins": [],
                            "outs": [],
                            "is_reset_sema": False,
                            "debug": ins.get("debug", 0),
                            "sync_info": {"on_update": [], "on_wait": [wi]},
                        })
                        changed = True
                    ins["sync_info"]["on_wait"] = [w[-1]]
                out.append(ins)
            bb["instructions"] = out
    return json.dumps(j).encode() if changed else bir_json


_patched = False


def _install_patch():
    global _patched
    if _patched:
        return
    _patched = True
    import concourse.bass_utils as bu
    import concourse.bass2jax as b2j

    orig = bu.compile_bir_kernel

    def patched(bir_json, tmpdir, neff_name="file.neff"):
        return orig(_split_multiwait(bir_json), tmpdir, neff_name)

    bu.compile_bir_kernel = patched
    b2j.compile_bir_kernel = patched


# ------------------------------------------------------------------ device


_KERNEL_CACHE = {}


def _build_nc(ngroups: int):
    """Blocks (psum, fully formed incl. gt-side bias): order [C1 W C2 H Sx Sy A]
      C1 = 2.5*Dcx, W = 1.25*Dw, C2 = 2.5*Dcy, H = 1.25*Dh,
      Sx = 1.25*(pw+gw), Sy = 1.25*(ph+gh), A = 6.25*(pa+ga)
    a_all = |[C1 W C2 H]|;  P = max(a1, aw), Q = max(a2, ah)   (2.5*P_t)
    iw,ih = relu([Sx Sy] - [P Q]); ew,eh = [Sx Sy] + [P Q]
    inter = iw*ih (6.25x), enc = ew*eh, U = A - inter
    [rU rE] = exp(-ln([U enc]));  [iou vv] = [inter U] * [rU rE]
    l1t = (a1+a2) + 2*(aw+ah)   (2.5*l1_t)
    F = CLS + l1t - iou - vv"""
    nc = bass.Bass(target_bir_lowering=False)
    f32, bf16 = mybir.dt.float32, mybir.dt.bfloat16
    lhsT = nc.dram_tensor("lhsT", [ngroups, KROWS, 128], bf16,
                          kind="ExternalInput")
    rhs = nc.dram_tensor("rhs", [ngroups, KROWS, NBLK, QPAD], bf16,
                         kind="ExternalInput")
    cls = nc.dram_tensor("cls", [ngroups, 128, QPAD], bf16,
                         kind="ExternalInput")
    out = nc.dram_tensor("out", [ngroups, 128, QPAD], bf16,
                         kind="ExternalOutput")

    def pv(ap2d, nb, w):
        """[128, nb*QC] slab -> 3D AP [128, nb, w] over the first w cols of
        each QC-wide slot."""
        v = ap2d.rearrange("p (b q) -> p b q", b=nb)
        return v[:, :, :w] if w != QC else v

    with TileContext(nc) as tc:
        with (
            tc.tile_pool(name="io", bufs=3) as io,
            tc.tile_pool(name="mid", bufs=4) as mid,
            tc.tile_pool(name="ps", bufs=2, space="PSUM") as psp,
        ):
            SL = 2 * QC  # 512-wide slots: elementwise ops batch chunk pairs

            def sv(ap2d, nb, w):
                v = ap2d.rearrange("p (b q) -> p b q", b=nb)
                return v[:, :, :w] if w != SL else v

            for g in range(ngroups):
                lt = io.tile([KROWS, 128], bf16, tag="lt", name="lt")
                rt = io.tile([KROWS, NBLK * QPAD], bf16, tag="rt", name="rt")
                ct = io.tile([128, QPAD], bf16, tag="ct", name="ct")
                nc.sync.dma_start(out=lt, in_=lhsT[g])
                rtv = rt.rearrange("k (b q) -> k b q", b=NBLK)
                for q0 in (0, 256, 512, 768):
                    nc.sync.dma_start(out=rtv[:, :, q0:q0 + 256],
                                      in_=rhs[g][:, :, q0:q0 + 256])
                nc.sync.dma_start(out=ct[:, 0:512], in_=cls[g][:, 0:512])
                nc.sync.dma_start(out=ct[:, 512:], in_=cls[g][:, 512:])
                rt3 = rt.rearrange("k (b q) -> k b q", b=NBLK)
                for (p0, p1) in ((0, 512), (512, 900)):
                    w = p1 - p0  # live width of this pair (512 or 388)

                    def t(tag, wdt=1):
                        return mid.tile([128, wdt * SL], bf16, tag=tag,
                                        name=tag)

                    # two 256-col psum chunks feed one 512-wide sbuf pair
                    aall = t("aall", 7)
                    a7 = aall.rearrange("p (b q) -> p b q", b=7)
                    for sub in (0, QC):
                        c0 = p0 + sub
                        c1 = min(c0 + QC, 900)
                        ws = c1 - c0
                        ps = psp.tile([128, NBLK * QC], f32, tag="ps",
                                      name="ps")
                        for j in range(NBLK):
                            nc.tensor.matmul(ps[:, j * QC:j * QC + ws],
                                             lt[:, :], rt3[:, j, c0:c1],
                                             start=True, stop=True)
                        # |.| of all 7 blocks in one op; [Sx Sy A] are
                        # positive so Abs doubles as the psum->bf16 copy.
                        nc.scalar.activation(
                            out=a7[:, :, sub:sub + ws],
                            in_=pv(ps[:, :7 * QC], 7, ws),
                            func=AF.Abs)
                    a1 = aall[:, 0 * SL:0 * SL + w]
                    aw_ = aall[:, 1 * SL:1 * SL + w]
                    a2 = aall[:, 2 * SL:2 * SL + w]
                    ah = aall[:, 3 * SL:3 * SL + w]
                    pq = t("pq", 2)
                    nc.vector.tensor_tensor(out=pq[:, :w], in0=a1, in1=aw_,
                                            op=AluOpType.max)
                    nc.vector.tensor_tensor(out=pq[:, SL:SL + w], in0=a2,
                                            in1=ah, op=AluOpType.max)
                    sxy = sv(aall[:, 4 * SL:6 * SL], 2, w)
                    pqv = sv(pq, 2, w)
                    dd = t("dd", 2)
                    nc.vector.tensor_tensor(out=sv(dd, 2, w), in0=sxy,
                                            in1=pqv, op=AluOpType.subtract)
                    ii = t("ii", 2)
                    nc.vector.tensor_scalar_max(out=sv(ii, 2, w),
                                                in0=sv(dd, 2, w), scalar1=0.0)
                    ee = t("ee", 2)
                    nc.vector.tensor_tensor(out=sv(ee, 2, w), in0=sxy,
                                            in1=pqv, op=AluOpType.add)
                    iue = t("iue", 3)  # slots: inter, U, enc
                    inter = iue[:, 0 * SL:0 * SL + w]
                    uslot = iue[:, 1 * SL:1 * SL + w]
                    encs = iue[:, 2 * SL:2 * SL + w]
                    nc.vector.tensor_tensor(out=inter, in0=ii[:, :w],
                                            in1=ii[:, SL:SL + w],
                                            op=AluOpType.mult)
                    nc.vector.tensor_tensor(out=encs, in0=ee[:, :w],
                                            in1=ee[:, SL:SL + w],
                                            op=AluOpType.mult)
                    nc.vector.tensor_tensor(
                        out=uslot, in0=aall[:, 6 * SL:6 * SL + w],
                        in1=inter, op=AluOpType.subtract)
                    rr = t("rr", 2)
                    nc.scalar.activation(out=sv(rr, 2, w),
                                         in_=sv(iue[:, SL:3 * SL], 2, w),
                                         func=AF.Ln)
                    nc.scalar.activation(out=sv(rr, 2, w), in_=sv(rr, 2, w),
                                         func=AF.Exp, scale=-1.0)
                    iv = t("iv", 2)
                    nc.vector.tensor_tensor(out=sv(iv, 2, w),
                                            in0=sv(iue[:, 0:2 * SL], 2, w),
                                            in1=sv(rr, 2, w),
                                            op=AluOpType.mult)
                    # l1t = (a1+a2) + 2*(aw+ah): s1 = [a1+a2 | aw+ah]
                    s1 = t("s1", 2)
                    nc.vector.tensor_tensor(out=sv(s1, 2, w),
                                            in0=sv(aall[:, 0:2 * SL], 2, w),
                                            in1=sv(aall[:, 2 * SL:4 * SL], 2, w),
                                            op=AluOpType.add)
                    # l1t = s1a + 2*s1b as two Pool adds (off critical path)
                    l1h = t("l1h")
                    nc.gpsimd.tensor_tensor(out=l1h[:, :w], in0=s1[:, :w],
                                            in1=s1[:, SL:SL + w],
                                            op=AluOpType.add)
                    l1t = t("l1t")
                    nc.gpsimd.tensor_tensor(out=l1t[:, :w], in0=l1h[:, :w],
                                            in1=s1[:, SL:SL + w],
                                            op=AluOpType.add)
                    zc, t4, ff = t("zc"), t("t4"), t("ff")
                    nc.gpsimd.tensor_tensor(out=zc[:, :w], in0=l1t[:, :w],
                                            in1=ct[:, p0:p1],
                                            op=AluOpType.add)
                    nc.vector.tensor_tensor(out=t4[:, :w], in0=iv[:, :w],
                                            in1=iv[:, SL:SL + w],
                                            op=AluOpType.add)
                    nc.gpsimd.tensor_tensor(out=ff[:, :w], in0=zc[:, :w],
                                            in1=t4[:, :w],
                                            op=AluOpType.subtract)
                    nc.sync.dma_start(out=out[g][:, p0:p1], in_=ff[:, :w])
    return nc


def _get_nc(ngroups: int):
    if ngroups not in _KERNEL_CACHE:
        _KERNEL_CACHE[ngroups] = _build_nc(ngroups)
    return _KERNEL_CACHE[ngroups]


# ------------------------------------------------------------------- host


def _hungarian(cost):
    # Jonker-Volgenant shortest-augmenting-path (mirrors the reference).
    cost = np.asarray(cost, dtype=np.float64)
    transposed = cost.shape[0] > cost.shape[1]
    C = cost.T if transposed else cost
    n, m = C.shape
    u = np.zeros(n + 1)
    v = np.zeros(m + 1)
    p = np.zeros(m + 1, dtype=np.int64)
    colr = np.arange(m + 1)
    for i in range(1, n + 1):
        p[0] = i
        j0 = 0
        minv = np.full(m + 1, np.inf)
        way = np.zeros(m + 1, dtype=np.int64)
        used = np.zeros(m + 1, dtype=bool)
        while True:
            used[j0] = True
            i0 = p[j0]
            unused = colr[1:][~used[1:]]
            cur = C[i0 - 1, unused - 1] - u[i0] - v[unused]
            better = cur < minv[unused]
            bidx = unused[better]
            minv[bidx] = cur[better]
            way[bidx] = j0
            j1 = unused[np.argmin(minv[unused])]
            delta = minv[j1]
            u[p[used]] += delta
            v[used] -= delta
            minv[~used] -= delta
            j0 = j1
            if p[j0] == 0:
                break
        while j0 != 0:
            jn = way[j0]
            p[j0] = p[jn]
            j0 = jn
    rows = p[1:] - 1
    cols = np.arange(m)
    sel = rows >= 0
    r, c = rows[sel], cols[sel]
    if transposed:
        r, c = c, r
    order = np.argsort(r)
    return r[order], c[order]


def _softmax_f32(x):
    x = x.astype(np.float32)
    m = x.max(axis=-1, keepdims=True)
    e = np.exp(x - m)
    return e / e.sum(axis=-1, keepdims=True)


def _bf16_split(x):
    hi = x.astype(np.float32).astype(BF16)
    lo = (x.astype(np.float32) - hi.astype(np.float32)).astype(BF16)
    return hi, lo


def _plan(nvalid):
    """Assign samples to cores (balanced by nvalid) and first-fit-decreasing
    pack whole samples into groups of <=128 gt rows / <=MAX_SLICES slices.
    Samples are never split across groups (the per-sample Hungarian needs
    the full cost matrix in one group)."""
    order = np.argsort(-nvalid, kind="stable")
    loads = [0] * NCORES
    core_samples = [[] for _ in range(NCORES)]
    for b in order:
        if nvalid[b] == 0:
            continue
        c = int(np.argmin(loads))
        loads[c] += int(nvalid[b])
        core_samples[c].append(int(b))
    plans = []
    for c in range(NCORES):
        bins = []  # list of [rows_used, slices]
        for b in core_samples[c]:  # already descending by nvalid
            n = int(nvalid[b])
            for bin_ in bins:
                if bin_[0] + n <= 128 and len(bin_[1]) < MAX_SLICES:
                    bin_[1].append((b, 0, n, bin_[0]))
                    bin_[0] += n
                    break
            else:
                bins.append([n, [(b, 0, n, 0)]])
        plans.append([bin_[1] for bin_ in bins])
    return plans


def kernel(pred_logits, pred_boxes, gt_classes, gt_boxes):
    _install_patch()
    pred_logits = np.asarray(pred_logits)
    pred_boxes = np.asarray(pred_boxes, dtype=np.float32)
    gt_classes = np.asarray(gt_classes)
    gt_boxes = np.asarray(gt_boxes, dtype=np.float32)
    Bv, Qv = pred_logits.shape[:2]

    prob = _softmax_f32(pred_logits)  # (B, Q, C+1)
    valid = gt_classes >= 0
    nvalid = valid.sum(1).astype(np.int64)
    plans = _plan(nvalid)
    ngroups = max((len(p) for p in plans), default=0)

    b_f = p_f = g_f = np.zeros(0, np.int64)
    if ngroups > 0:
        # pred-side rows per block, shared per sample; block order
        # [C1 W C2 H Sx Sy A]
        pcx, pcy = pred_boxes[..., 0], pred_boxes[..., 1]
        pw, ph = pred_boxes[..., 2], pred_boxes[..., 3]
        pa = pw * ph
        pred_rows = np.stack(  # (B, NBLK, Q)
            [2.5 * pcx, 1.25 * pw, 2.5 * pcy, 1.25 * ph,
             1.25 * pw, 1.25 * ph, 6.25 * pa], axis=1)
        # gt-side bias value per block (full G; sliced per group)
        gcx, gcy = gt_boxes[..., 0], gt_boxes[..., 1]
        gw, gh = gt_boxes[..., 2], gt_boxes[..., 3]
        ga = gw * gh
        bias_all = np.stack(  # (B, NBLK, G)
            [-2.5 * gcx, -1.25 * gw, -2.5 * gcy, -1.25 * gh,
             1.25 * gw, 1.25 * gh, 6.25 * ga], axis=1).astype(np.float32)

        lhsT = np.zeros((NCORES, ngroups, KROWS, 128), BF16)
        rhs = np.zeros((NCORES, ngroups, KROWS, NBLK, QPAD), BF16)
        clsb = np.zeros((NCORES, ngroups, 128, QPAD), BF16)
        pr_hi = {}
        for c in range(NCORES):
            for g, slices in enumerate(plans[c]):
                for si, (b, vstart, ln, poff) in enumerate(slices):
                    vidx = np.nonzero(valid[b])[0][vstart:vstart + ln]
                    if b not in pr_hi:
                        pr_hi[b] = _bf16_split(pred_rows[b])
                    hi, lo = pr_hi[b]  # (NBLK, Q)
                    lhsT[c, g, si, poff:poff + ln] = 1.0
                    lhsT[c, g, 7 + si, poff:poff + ln] = 1.0
                    rhs[c, g, si, :, :Qv] = hi
                    rhs[c, g, 7 + si, :, :Qv] = lo
                    bhi, blo = _bf16_split(bias_all[b, :, vidx])  # (ln, NBLK)
                    lhsT[c, g, 14:21, poff:poff + ln] = bhi.T
                    lhsT[c, g, 21:28, poff:poff + ln] = blo.T
                    clsb[c, g, poff:poff + ln, :Qv] = \
                        -prob[b][:, gt_classes[b, vidx]].T
                for j in range(NBLK):
                    rhs[c, g, 14 + j, j, :] = 1.0
                    rhs[c, g, 21 + j, j, :] = 1.0
        in_maps = [{"lhsT": lhsT[c], "rhs": rhs[c],
                    "cls": clsb[c]} for c in range(NCORES)]
        nc = _get_nc(ngroups)
        res = run_bass_kernel_spmd(nc, in_maps, core_ids=list(range(NCORES)))

        bs, ps_, gs = [], [], []
        for c in range(NCORES):
            outc = res.results[c]["out"]  # (ngroups, 128, QPAD)
            for g, slices in enumerate(plans[c]):
                for (b, vstart, ln, poff) in slices:
                    Fm = outc[g, poff:poff + ln, :Qv].astype(np.float32)
                    vidx = np.nonzero(valid[b])[0][vstart:vstart + ln]
                    r, q = _hungarian(Fm)  # rows=gt-slice, cols=query
                    bs.append(np.full(r.size, b, dtype=np.int64))
                    ps_.append(q.astype(np.int64))
                    gs.append(vidx[r].astype(np.int64))
        if bs:
            b_f = np.concatenate(bs)
            p_f = np.concatenate(ps_)
            g_f = np.concatenate(gs)

    # --- final loss on host (mirrors reference.detection_loss) ---
    n_matched = max(int(b_f.size), 1)
    tc = np.full((Bv, Qv), NUM_CLASSES, dtype=np.int64)
    if b_f.size:
        tc[b_f, p_f] = gt_classes[b_f, g_f]
    logits = pred_logits.astype(np.float64)
    m = logits.max(-1, keepdims=True)
    lse = m + np.log(np.exp(logits - m).sum(-1, keepdims=True))
    logp = (logits - lse).reshape(-1, C1)
    tt = tc.reshape(-1)
    w = np.ones(C1)
    w[-1] = 0.1
    nll = -logp[np.arange(tt.size), tt]
    wt = w[tt]
    cls_loss = (wt * nll).sum() / wt.sum()
    if b_f.size:
        mp = pred_boxes[b_f, p_f].astype(np.float64)
        mg = gt_boxes[b_f, g_f].astype(np.float64)
        l1 = np.abs(mp - mg).sum()
        mpx = np.stack([mp[:, 0] - mp[:, 2] / 2, mp[:, 1] - mp[:, 3] / 2,
                        mp[:, 0] + mp[:, 2] / 2, mp[:, 1] + mp[:, 3] / 2], -1)
        mgx = np.stack([mg[:, 0] - mg[:, 2] / 2, mg[:, 1] - mg[:, 3] / 2,
                        mg[:, 0] + mg[:, 2] / 2, mg[:, 1] + mg[:, 3] / 2], -1)
        a1 = np.clip(mpx[:, 2] - mpx[:, 0], 0, None) * \
            np.clip(mpx[:, 3] - mpx[:, 1], 0, None)
        a2 = np.clip(mgx[:, 2] - mgx[:, 0], 0, None) * \
            np.clip(mgx[:, 3] - mgx[:, 1], 0, None)
        lt = np.maximum(mpx[:, :2], mgx[:, :2])
        rb = np.minimum(mpx[:, 2:], mgx[:, 2:])
        wh = np.clip(rb - lt, 0, None)
        inter = wh[:, 0] * wh[:, 1]
        union = a1 + a2 - inter
        iou = inter / np.clip(union, 1e-6, None)
        lte = np.minimum(mpx[:, :2], mgx[:, :2])
        rbe = np.maximum(mpx[:, 2:], mgx[:, 2:])
        whe = np.clip(rbe - lte, 0, None)
        encl = whe[:, 0] * whe[:, 1]
        gd = iou - (encl - union) / np.clip(encl, 1e-6, None)
        gl = (1.0 - gd).sum()
    else:
        l1 = 0.0
        gl = 0.0
    total = CLS_W * cls_loss + L1_W * l1 / n_matched + GIOU_W * gl / n_matched
    return np.float32(total)
